# revision 13
# baseline (speedup 1.0000x reference)
"""Trainium2 Bass kernel: 3x3 stride-1 VALID conv (NHWC, HWIO) + bias + ReLU.

Problem shapes:
  x       (32, 112, 112, 64)  f32
  kernels (3, 3, 64, 128)     f32
  biases  (128,)              f32
  out     (32, 110, 110, 128) f32

Strategy:
  * Data-parallel: 4 images per core across 8 NeuronCores (no collectives).
  * Host pre-packs x into a channel/row-parity-major layout
      X[p, rp, b, w]  with p = (h%2)*64 + c,  rp = h//2
    so adjacent image rows sit on opposite halves of the 128 SBUF
    partitions. A 3x3x64 conv then becomes PSUM-accumulated matmuls
    per output row: 3 kw shifts x {one K=128 row-pair matmul + one K=64
    single-row matmul}, batching all 4 images into N=440 moving columns.
  * The K=64 leftovers of an even output row live on partitions 0:64
    (T0 row-tile) and of the odd row on 64:128 (T8). Issuing them
    pc-interleaved (A B A B A B) makes the two 64x128 row-tiles stream
    concurrently, so the 6 half matmuls cost ~3 slots instead of 6:
    9 x 440-col slots per 2 output rows = the fp16 PE roofline.
  * 2 a-iters per phase group to halve 128<->64 tile-mode switches
    (each mode change drains the PE array).
  * fp16 operands: 2-byte LDWEIGHTS fast path, full 1 col/cycle stream
    rate, fp32 PSUM accumulation.
  * ScalarE fuses bias+ReLU on the PSUM->SBUF evacuation, writing fp16
    (halves the output DMA bytes; host upcasts to fp32).
  * Host restores NHWC from the [f, h, b, w] device output layout.
"""

import numpy as np

import concourse.bass as bass
import concourse.mybir as mybir
from concourse import bacc
from concourse.bass_utils import run_bass_kernel_spmd
from concourse.tile import TileContext

N_CORES = 8
B = 4  # images per core
H = W = 112
C = 64
F = 128
KH = KW = 3
HO = WO = 110
NRP = H // 2  # 56 row pairs per image
A = HO // 2  # 55 output row-parity iterations

F32 = mybir.dt.float32
F16 = mybir.dt.float16
MM_DTYPE = F16
OUT_DTYPE = F16

X_ELEMS = NRP * B * W  # per-partition input elements (25088)
O_ELEMS = A * 2 * B * WO  # per-partition output elements (48400)

_TRACE = False
LAST_RESULT = None
_NC_CACHE = None


def _build_bass():
    nc = bacc.Bacc("TRN2", target_bir_lowering=False, debug=False)
    x_d = nc.dram_tensor("x", [128, X_ELEMS], MM_DTYPE, kind="ExternalInput")
    # weights (9 stacked [128,128] lhsT tiles) + fp32 bias packed as the
    # last two fp16 columns (bitcast back to f32 on device)
    w_d = nc.dram_tensor("w", [128, 9 * F + 2], MM_DTYPE, kind="ExternalInput")
    o_d = nc.dram_tensor("o", [128, O_ELEMS], OUT_DTYPE, kind="ExternalOutput")

    rpw = B * W  # elems per rowpair per partition (448)
    ow = 2 * B * WO  # output elems per a-iteration (880)

    with TileContext(nc) as tc:
        with (
            tc.tile_pool(name="xres", bufs=1) as xpool,
            tc.tile_pool(name="wpool", bufs=1) as wpool,
            tc.tile_pool(name="psum", bufs=8, space="PSUM") as ppool,
            tc.tile_pool(name="opool", bufs=4) as opool,
        ):
            # Scalar ring: runs concurrently with chunk 0 on SWDGE.
            wt = wpool.tile([128, 9 * F + 2], MM_DTYPE)
            nc.sync.dma_start(out=wt[:], in_=w_d[:])
            bt = wt[:, 9 * F : 9 * F + 2].bitcast(F32)

            # Warm-up scratch, zeroed on the otherwise-idle VectorE so the
            # PE pre-warm isn't queued behind the gpsimd SWDGE descriptor
            # generation for the input chunks (~630ns each, serial).
            warm = wpool.tile([128, 440], MM_DTYPE)
            nc.vector.memset(warm[:], 0.0)

            # Fast-start chunk schedule: small chunks first so the first
            # matmul group can begin ASAP, larger chunks once compute is
            # the slower consumer. Input chunks ride SWDGE (gpsimd) whose
            # DMASW sem lanes are disjoint from the DMAHW lanes used by
            # output DMAs -- and whose Q7 descriptor generation does not
            # contend with the sync/scalar HWDGE trigger processing.
            chunk_rps = [1, 1, 2, 4] + [8] * 6
            assert sum(chunk_rps) == NRP
            rp2view = []  # rowpair -> (view, local index)
            for ch, nrp_ch in enumerate(chunk_rps):
                cht = xpool.tile([128, nrp_ch * rpw], MM_DTYPE, tag=f"xch{ch}")
                s = len(rp2view) * rpw
                if ch == 0:
                    # Chunk 0 rides the otherwise-idle scalar HWDGE queue
                    # (~0.6us latency, RTL descriptor gen) so the first real
                    # matmul isn't held ~2us by the SWDGE fixed cost.
                    nc.scalar.dma_start(out=cht[:], in_=x_d[:, s : s + nrp_ch * rpw])
                else:
                    nc.gpsimd.dma_start(out=cht[:], in_=x_d[:, s : s + nrp_ch * rpw])
                v = cht[:].rearrange("p (rp b w) -> p rp b w", rp=nrp_ch, b=B, w=W)
                for r in range(nrp_ch):
                    rp2view.append((v, r))

            def xs(lo, hi, rp, kw):
                v, r = rp2view[rp]
                return v[lo:hi, r, :, kw : kw + WO]

            wv = wt[:, 0 : 9 * F].rearrange("p (i f) -> p i f", i=9, f=F)

            # PE pre-warm: ~1.6us of dummy matmuls on zeroed SBUF while the
            # first input chunks are still in flight. Keeps the PE busy so
            # the PE_HAM clock gate (cold = 1.2GHz) releases as early as
            # possible; the scratch PSUM bank is never read.
            wps = ppool.tile([128, B * WO], F32, tag="ps")
            NWARM = 4
            for j in range(NWARM):
                nc.tensor.matmul(
                    wps[:], warm[:, 0:128], warm[:, 0:440], start=(j == 0),
                    stop=(j == NWARM - 1),
                )

            # Adaptive output DMA grouping: big groups steady-state, small
            # at the end so the final (unoverlapped) store is short.
            dma_grps = [8] * 6 + [4, 2, 1]
            assert sum(dma_grps) == A
            a2dma = []  # a -> (group index, offset in group, group size, a0)
            a0 = 0
            for gi, n in enumerate(dma_grps):
                for j in range(n):
                    a2dma.append((gi, j, n, a0))
                a0 += n

            AG = 4  # a-iterations per PE phase group (amortize tile-mode switches)
            ot = None

            def emit_fulls(ais, psA, psB):
                # All full K=128 row-pair matmuls of the group (128x128 mode).
                for a in ais:
                    pa = psA[a][:].rearrange("p (b w) -> p b w", b=B)
                    pb = psB[a][:].rearrange("p (b w) -> p b w", b=B)
                    for kw in range(KW):
                        # out row 2a: kh=0,1 -> rows 2a,2a+1 = rowpair a,
                        # weights [k0;k1].
                        nc.tensor.matmul(
                            pa, wv[:, kw, :], xs(0, 128, a, kw),
                            start=(kw == 0 and a not in started),
                            stop=(a in started and kw == KW - 1),
                        )
                    for kw in range(KW):
                        # out row 2a+1: kh=1,2 -> rows 2a+2,2a+3 = rowpair
                        # a+1, weights [k1;k2].
                        nc.tensor.matmul(
                            pb, wv[:, 3 + kw, :], xs(0, 128, a + 1, kw),
                            start=(kw == 0 and a not in started),
                            stop=(a in started and kw == KW - 1),
                        )
                    if a in started:
                        started.discard(a)
                        evacuate(a, psA, psB)
                    else:
                        started.add(a)

            def emit_pairs(ais, psA, psB):
                # K=64 leftovers as pc-interleaved T0/T8 row-tile pairs
                # (64x128 mode) so the two halves stream concurrently.
                for a in ais:
                    pa = psA[a][:].rearrange("p (b w) -> p b w", b=B)
                    pb = psB[a][:].rearrange("p (b w) -> p b w", b=B)
                    for kw in range(KW):
                        last = a in started and kw == KW - 1
                        # out row 2a: kh=2 -> row 2a+2 (low half of rowpair
                        # a+1), weights k2 (= low half of tile 6+kw). T0.
                        nc.tensor.matmul(
                            pa, wv[0:64, 6 + kw, :], xs(0, 64, a + 1, kw),
                            start=(kw == 0 and a not in started), stop=last,
                        )
                        # out row 2a+1: kh=0 -> row 2a+1 (high half of
                        # rowpair a), weights k0 (high half of 6+kw). T8.
                        nc.tensor.matmul(
                            pb, wv[64:128, 6 + kw, :], xs(64, 128, a, kw),
                            start=(kw == 0 and a not in started), stop=last,
                        )
                    if a in started:
                        started.discard(a)
                        evacuate(a, psA, psB)
                    else:
                        started.add(a)

            def evacuate(a, psA, psB):
                # Bias+ReLU evacuation, fp32 PSUM -> fp16 SBUF, split
                # across ScalarE (ACT) and VectorE (fused add+max) so the
                # two rows drain in parallel.
                nonlocal ot
                gi, ji, n_in_g, ga0 = a2dma[a]
                if ji == 0:
                    ot = opool.tile([128, n_in_g * ow], OUT_DTYPE, tag="ot")
                o0 = ot[:, (ji * 2) * B * WO : (ji * 2 + 1) * B * WO]
                o1 = ot[:, (ji * 2 + 1) * B * WO : (ji * 2 + 2) * B * WO]
                nc.scalar.activation(
                    out=o0, in_=psA[a][:],
                    func=mybir.ActivationFunctionType.Relu, bias=bt,
                )
                nc.vector.tensor_scalar(
                    o1, psB[a][:], bt, 0.0,
                    mybir.AluOpType.add, mybir.AluOpType.max,
                )
                if a == A - 1:
                    # Final a-iteration: ship each row on its own HWDGE
                    # queue the moment its evacuation lands, so the last
                    # (unoverlapped) transfer is one 110-col row and the
                    # two triggers process in parallel.
                    hw = B * WO
                    nc.sync.dma_start(
                        out=o_d[:, (ga0 + ji) * ow : (ga0 + ji) * ow + hw],
                        in_=o0,
                    )
                    nc.scalar.dma_start(
                        out=o_d[:, (ga0 + ji) * ow + hw : (ga0 + ji + 1) * ow],
                        in_=o1,
                    )
                elif ji == n_in_g - 1:
                    # Idle sync-queue HWDGE: output-DMA triggers don't queue
                    # behind ACT work on the scalar engine.
                    nc.sync.dma_start(
                        out=o_d[:, ga0 * ow : (ga0 + n_in_g) * ow], in_=ot[:]
                    )

            # Alternate fulls/pairs order per group so consecutive groups
            # share the PE tiling mode at the boundary: ..fulls | pairs ::
            # pairs | fulls.. -> one 128<->64 mode drain per group, not two.
            started = set()
            groups = [list(range(g0, min(g0 + AG, A))) for g0 in range(0, A, AG)]
            for gidx, ais in enumerate(groups):
                psA = {}
                psB = {}
                for a in ais:
                    psA[a] = ppool.tile([128, B * WO], F32, name="psA", tag="ps")
                    psB[a] = ppool.tile([128, B * WO], F32, name="psB", tag="ps")
                if gidx % 2 == 0:
                    emit_fulls(ais, psA, psB)
                    emit_pairs(ais, psA, psB)
                else:
                    emit_pairs(ais, psA, psB)
                    emit_fulls(ais, psA, psB)
    nc.compile()
    return nc


def _prep_weights(kernels, biases):
    k = np.asarray(kernels, np.float32)  # (3,3,64,128) HWIO
    ws = []
    for kw in range(KW):  # [k0;k1] pairs (even rows, kh=0/1)
        ws.append(np.concatenate([k[0, kw], k[1, kw]], axis=0))
    for kw in range(KW):  # [k1;k2] pairs (odd rows, kh=1/2)
        ws.append(np.concatenate([k[1, kw], k[2, kw]], axis=0))
    for kw in range(KW):  # [k2;k0]: k2 low half (even kh=2), k0 high (odd kh=0)
        ws.append(np.concatenate([k[2, kw], k[0, kw]], axis=0))
    wdev = np.stack(ws, axis=1).reshape(128, 9 * F).astype(np.float16)
    # fp32 bias bits carried as two fp16 columns (device bitcasts back)
    bdev = np.asarray(biases, np.float32).reshape(128, 1).view(np.float16)
    return np.ascontiguousarray(np.concatenate([wdev, bdev], axis=1))


def kernel(**inputs):
    global _NC_CACHE, LAST_RESULT
    x = np.asarray(inputs["x"], np.float32).astype(np.float16)
    wdev = _prep_weights(inputs["kernels"], inputs["biases"])

    if _NC_CACHE is None:
        _NC_CACHE = _build_bass()
    nc = _NC_CACHE

    in_maps = []
    for i in range(N_CORES):
        xc = x[i * B : (i + 1) * B]  # [4,112,112,64]
        # [b, rp, par, w, c] -> [par, c, rp, b, w]; partition p = par*64 + c
        xp = xc.reshape(B, NRP, 2, W, C).transpose(2, 4, 1, 0, 3)
        in_maps.append(
            {"x": np.ascontiguousarray(xp).reshape(128, X_ELEMS), "w": wdev}
        )

    LAST_RESULT = run_bass_kernel_spmd(
        nc, in_maps, core_ids=list(range(N_CORES)), trace=_TRACE
    )

    outs = []
    for res in LAST_RESULT.results:
        o = res["o"].astype(np.float32).reshape(F, A, 2, B, WO).transpose(3, 1, 2, 4, 0)
        outs.append(o.reshape(B, HO, WO, F))
    return np.ascontiguousarray(np.concatenate(outs, axis=0))


# revision 14
# speedup vs baseline: 1.0077x; 1.0077x over previous
"""Trainium2 Bass kernel: 3x3 stride-1 VALID conv (NHWC, HWIO) + bias + ReLU.

Problem shapes:
  x       (32, 112, 112, 64)  f32
  kernels (3, 3, 64, 128)     f32
  biases  (128,)              f32
  out     (32, 110, 110, 128) f32

Strategy:
  * Data-parallel: 4 images per core across 8 NeuronCores (no collectives).
  * Host pre-packs x into a channel/row-parity-major layout
      X[p, rp, b, w]  with p = (h%2)*64 + c,  rp = h//2
    so adjacent image rows sit on opposite halves of the 128 SBUF
    partitions. A 3x3x64 conv then becomes PSUM-accumulated matmuls
    per output row: 3 kw shifts x {one K=128 row-pair matmul + one K=64
    single-row matmul}, batching all 4 images into N=440 moving columns.
  * The K=64 leftovers of an even output row live on partitions 0:64
    (T0 row-tile) and of the odd row on 64:128 (T8). Issuing them
    pc-interleaved (A B A B A B) makes the two 64x128 row-tiles stream
    concurrently, so the 6 half matmuls cost ~3 slots instead of 6:
    9 x 440-col slots per 2 output rows = the fp16 PE roofline.
  * 2 a-iters per phase group to halve 128<->64 tile-mode switches
    (each mode change drains the PE array).
  * fp16 operands: 2-byte LDWEIGHTS fast path, full 1 col/cycle stream
    rate, fp32 PSUM accumulation.
  * ScalarE fuses bias+ReLU on the PSUM->SBUF evacuation, writing fp16
    (halves the output DMA bytes; host upcasts to fp32).
  * Host restores NHWC from the [f, h, b, w] device output layout.
"""

import numpy as np

import concourse.bass as bass
import concourse.mybir as mybir
from concourse import bacc
from concourse.bass_utils import run_bass_kernel_spmd
from concourse.tile import TileContext

N_CORES = 8
B = 4  # images per core
H = W = 112
C = 64
F = 128
KH = KW = 3
HO = WO = 110
NRP = H // 2  # 56 row pairs per image
A = HO // 2  # 55 output row-parity iterations

F32 = mybir.dt.float32
F16 = mybir.dt.float16
MM_DTYPE = F16
OUT_DTYPE = F16

X_ELEMS = NRP * B * W  # per-partition input elements (25088)
O_ELEMS = A * 2 * B * WO  # per-partition output elements (48400)

_TRACE = False
LAST_RESULT = None
_NC_CACHE = None


def _build_bass():
    nc = bacc.Bacc("TRN2", target_bir_lowering=False, debug=False)
    x_d = nc.dram_tensor("x", [128, X_ELEMS], MM_DTYPE, kind="ExternalInput")
    # weights (9 stacked [128,128] lhsT tiles) + fp32 bias packed as the
    # last two fp16 columns (bitcast back to f32 on device)
    w_d = nc.dram_tensor("w", [128, 9 * F + 2], MM_DTYPE, kind="ExternalInput")
    o_d = nc.dram_tensor("o", [128, O_ELEMS], OUT_DTYPE, kind="ExternalOutput")

    rpw = B * W  # elems per rowpair per partition (448)
    ow = 2 * B * WO  # output elems per a-iteration (880)

    with TileContext(nc) as tc:
        with (
            tc.tile_pool(name="xres", bufs=1) as xpool,
            tc.tile_pool(name="wpool", bufs=1) as wpool,
            tc.tile_pool(name="psum", bufs=8, space="PSUM") as ppool,
            tc.tile_pool(name="opool", bufs=4) as opool,
        ):
            # Scalar ring: runs concurrently with chunk 0 on SWDGE.
            wt = wpool.tile([128, 9 * F + 2], MM_DTYPE)
            nc.sync.dma_start(out=wt[:], in_=w_d[:])
            bt = wt[:, 9 * F : 9 * F + 2].bitcast(F32)

            # Warm-up scratch, zeroed on the otherwise-idle VectorE so the
            # PE pre-warm isn't queued behind the gpsimd SWDGE descriptor
            # generation for the input chunks (~630ns each, serial).
            warm = wpool.tile([128, 440], MM_DTYPE)
            nc.vector.memset(warm[:], 0.0)

            # Fast-start chunk schedule: small chunks first so the first
            # matmul group can begin ASAP, larger chunks once compute is
            # the slower consumer. Input chunks ride SWDGE (gpsimd) whose
            # DMASW sem lanes are disjoint from the DMAHW lanes used by
            # output DMAs -- and whose Q7 descriptor generation does not
            # contend with the sync/scalar HWDGE trigger processing.
            chunk_rps = [1, 1, 2, 4] + [8] * 6
            assert sum(chunk_rps) == NRP
            rp2view = []  # rowpair -> (view, local index)
            for ch, nrp_ch in enumerate(chunk_rps):
                cht = xpool.tile([128, nrp_ch * rpw], MM_DTYPE, tag=f"xch{ch}")
                s = len(rp2view) * rpw
                if ch == 0:
                    # Chunk 0 rides the otherwise-idle scalar HWDGE queue
                    # (~0.6us latency, RTL descriptor gen) so the first real
                    # matmul isn't held ~2us by the SWDGE fixed cost.
                    nc.scalar.dma_start(out=cht[:], in_=x_d[:, s : s + nrp_ch * rpw])
                else:
                    nc.gpsimd.dma_start(out=cht[:], in_=x_d[:, s : s + nrp_ch * rpw])
                v = cht[:].rearrange("p (rp b w) -> p rp b w", rp=nrp_ch, b=B, w=W)
                for r in range(nrp_ch):
                    rp2view.append((v, r))

            def xs(lo, hi, rp, kw):
                v, r = rp2view[rp]
                return v[lo:hi, r, :, kw : kw + WO]

            wv = wt[:, 0 : 9 * F].rearrange("p (i f) -> p i f", i=9, f=F)

            # PE pre-warm: ~1.6us of dummy matmuls on zeroed SBUF while the
            # first input chunks are still in flight. Keeps the PE busy so
            # the PE_HAM clock gate (cold = 1.2GHz) releases as early as
            # possible; the scratch PSUM bank is never read.
            wps = ppool.tile([128, B * WO], F32, tag="ps")
            NWARM = 6
            for j in range(NWARM):
                nc.tensor.matmul(
                    wps[:], warm[:, 0:128], warm[:, 0:440], start=(j == 0),
                    stop=(j == NWARM - 1),
                )

            # Adaptive output DMA grouping: big groups steady-state, small
            # at the end so the final (unoverlapped) store is short.
            dma_grps = [8] * 6 + [4, 2, 1]
            assert sum(dma_grps) == A
            a2dma = []  # a -> (group index, offset in group, group size, a0)
            a0 = 0
            for gi, n in enumerate(dma_grps):
                for j in range(n):
                    a2dma.append((gi, j, n, a0))
                a0 += n

            AG = 4  # a-iterations per PE phase group (amortize tile-mode switches)
            ot = None

            def emit_fulls(ais, psA, psB):
                # All full K=128 row-pair matmuls of the group (128x128 mode).
                for a in ais:
                    pa = psA[a][:].rearrange("p (b w) -> p b w", b=B)
                    pb = psB[a][:].rearrange("p (b w) -> p b w", b=B)
                    for kw in range(KW):
                        # out row 2a: kh=0,1 -> rows 2a,2a+1 = rowpair a,
                        # weights [k0;k1].
                        nc.tensor.matmul(
                            pa, wv[:, kw, :], xs(0, 128, a, kw),
                            start=(kw == 0 and a not in started),
                            stop=(a in started and kw == KW - 1),
                        )
                    for kw in range(KW):
                        # out row 2a+1: kh=1,2 -> rows 2a+2,2a+3 = rowpair
                        # a+1, weights [k1;k2].
                        nc.tensor.matmul(
                            pb, wv[:, 3 + kw, :], xs(0, 128, a + 1, kw),
                            start=(kw == 0 and a not in started),
                            stop=(a in started and kw == KW - 1),
                        )
                    if a in started:
                        started.discard(a)
                        evacuate(a, psA, psB)
                    else:
                        started.add(a)

            def emit_pairs(ais, psA, psB):
                # K=64 leftovers as pc-interleaved T0/T8 row-tile pairs
                # (64x128 mode) so the two halves stream concurrently.
                for a in ais:
                    pa = psA[a][:].rearrange("p (b w) -> p b w", b=B)
                    pb = psB[a][:].rearrange("p (b w) -> p b w", b=B)
                    for kw in range(KW):
                        last = a in started and kw == KW - 1
                        # out row 2a: kh=2 -> row 2a+2 (low half of rowpair
                        # a+1), weights k2 (= low half of tile 6+kw). T0.
                        nc.tensor.matmul(
                            pa, wv[0:64, 6 + kw, :], xs(0, 64, a + 1, kw),
                            start=(kw == 0 and a not in started), stop=last,
                        )
                        # out row 2a+1: kh=0 -> row 2a+1 (high half of
                        # rowpair a), weights k0 (high half of 6+kw). T8.
                        nc.tensor.matmul(
                            pb, wv[64:128, 6 + kw, :], xs(64, 128, a, kw),
                            start=(kw == 0 and a not in started), stop=last,
                        )
                    if a in started:
                        started.discard(a)
                        evacuate(a, psA, psB)
                    else:
                        started.add(a)

            def evacuate(a, psA, psB):
                # Bias+ReLU evacuation, fp32 PSUM -> fp16 SBUF, split
                # across ScalarE (ACT) and VectorE (fused add+max) so the
                # two rows drain in parallel.
                nonlocal ot
                gi, ji, n_in_g, ga0 = a2dma[a]
                if ji == 0:
                    ot = opool.tile([128, n_in_g * ow], OUT_DTYPE, tag="ot")
                o0 = ot[:, (ji * 2) * B * WO : (ji * 2 + 1) * B * WO]
                o1 = ot[:, (ji * 2 + 1) * B * WO : (ji * 2 + 2) * B * WO]
                nc.scalar.activation(
                    out=o0, in_=psA[a][:],
                    func=mybir.ActivationFunctionType.Relu, bias=bt,
                )
                nc.vector.tensor_scalar(
                    o1, psB[a][:], bt, 0.0,
                    mybir.AluOpType.add, mybir.AluOpType.max,
                )
                if a == A - 1:
                    # Final a-iteration: ship each row on its own HWDGE
                    # queue the moment its evacuation lands, so the last
                    # (unoverlapped) transfer is one 110-col row and the
                    # two triggers process in parallel.
                    hw = B * WO
                    nc.sync.dma_start(
                        out=o_d[:, (ga0 + ji) * ow : (ga0 + ji) * ow + hw],
                        in_=o0,
                    )
                    nc.scalar.dma_start(
                        out=o_d[:, (ga0 + ji) * ow + hw : (ga0 + ji + 1) * ow],
                        in_=o1,
                    )
                elif ji == n_in_g - 1:
                    # Idle sync-queue HWDGE: output-DMA triggers don't queue
                    # behind ACT work on the scalar engine.
                    nc.sync.dma_start(
                        out=o_d[:, ga0 * ow : (ga0 + n_in_g) * ow], in_=ot[:]
                    )

            # Alternate fulls/pairs order per group so consecutive groups
            # share the PE tiling mode at the boundary: ..fulls | pairs ::
            # pairs | fulls.. -> one 128<->64 mode drain per group, not two.
            started = set()
            groups = [list(range(g0, min(g0 + AG, A))) for g0 in range(0, A, AG)]
            for gidx, ais in enumerate(groups):
                psA = {}
                psB = {}
                for a in ais:
                    psA[a] = ppool.tile([128, B * WO], F32, name="psA", tag="ps")
                    psB[a] = ppool.tile([128, B * WO], F32, name="psB", tag="ps")
                if gidx % 2 == 0:
                    emit_fulls(ais, psA, psB)
                    emit_pairs(ais, psA, psB)
                else:
                    emit_pairs(ais, psA, psB)
                    emit_fulls(ais, psA, psB)
    nc.compile()
    return nc


def _prep_weights(kernels, biases):
    k = np.asarray(kernels, np.float32)  # (3,3,64,128) HWIO
    ws = []
    for kw in range(KW):  # [k0;k1] pairs (even rows, kh=0/1)
        ws.append(np.concatenate([k[0, kw], k[1, kw]], axis=0))
    for kw in range(KW):  # [k1;k2] pairs (odd rows, kh=1/2)
        ws.append(np.concatenate([k[1, kw], k[2, kw]], axis=0))
    for kw in range(KW):  # [k2;k0]: k2 low half (even kh=2), k0 high (odd kh=0)
        ws.append(np.concatenate([k[2, kw], k[0, kw]], axis=0))
    wdev = np.stack(ws, axis=1).reshape(128, 9 * F).astype(np.float16)
    # fp32 bias bits carried as two fp16 columns (device bitcasts back)
    bdev = np.asarray(biases, np.float32).reshape(128, 1).view(np.float16)
    return np.ascontiguousarray(np.concatenate([wdev, bdev], axis=1))


def kernel(**inputs):
    global _NC_CACHE, LAST_RESULT
    x = np.asarray(inputs["x"], np.float32).astype(np.float16)
    wdev = _prep_weights(inputs["kernels"], inputs["biases"])

    if _NC_CACHE is None:
        _NC_CACHE = _build_bass()
    nc = _NC_CACHE

    in_maps = []
    for i in range(N_CORES):
        xc = x[i * B : (i + 1) * B]  # [4,112,112,64]
        # [b, rp, par, w, c] -> [par, c, rp, b, w]; partition p = par*64 + c
        xp = xc.reshape(B, NRP, 2, W, C).transpose(2, 4, 1, 0, 3)
        in_maps.append(
            {"x": np.ascontiguousarray(xp).reshape(128, X_ELEMS), "w": wdev}
        )

    LAST_RESULT = run_bass_kernel_spmd(
        nc, in_maps, core_ids=list(range(N_CORES)), trace=_TRACE
    )

    outs = []
    for res in LAST_RESULT.results:
        o = res["o"].astype(np.float32).reshape(F, A, 2, B, WO).transpose(3, 1, 2, 4, 0)
        outs.append(o.reshape(B, HO, WO, F))
    return np.ascontiguousarray(np.concatenate(outs, axis=0))
